# revision 41
# baseline (speedup 1.0000x reference)
"""Trainium2 Bass kernel for the Dinomaly anomaly head (ViTill fuse + bottleneck
MLP + 8 linear-attention decoder blocks + feature-map assembly).

Sharding: sequence-parallel over the 4096 (batch x token) positions across 8
cores (512 tokens each; core c owns batch c//2, token half c%2). Params are
replicated. The only cross-core dependency is the linear-attention KV/k-sum
statistic, which is summed over a batch's full 1024 tokens: each core computes
its partial [16,64,65] stat and a pair-wise AllReduce ([0,1],[2,3],...)
combines the two halves.

Layout: activations are feature-major [C, T] on-chip so per-feature params
broadcast along the free dim; LayerNorm stats (sums over C, a cross-partition
reduction) run on the PE via ones-vector matmuls, and per-token scalars are
broadcast across partitions via tiny PE matmuls.

Precision: matmuls in bf16 (fp32 PSUM accumulate), residual stream in fp32r
(fp32 with 11-bit mantissa, required so the LN-stat matmuls can consume it at
full PE rate), LN/score scalar math in fp32.

Engine split: PE does all matmuls (including LN stat/broadcast); DVE handles
critical-path elementwise (LN normalize, elu min/add, residual adds); the
scalar engine does exp/relu/gelu/sqrt plus x^2 for LN stats and PSUM-drain
copies; gpsimd (Pool) takes SBUF-side non-critical work (de accumulators,
en/de map scaling, kv-stat unpack) since it has no PSUM port.
"""

import os

import ml_dtypes
import numpy as np

import concourse.bass as bass
import concourse.mybir as mybir
import concourse.tile as tile
from concourse import bacc
from concourse.bass_utils import run_bass_kernel_spmd

F32 = mybir.dt.float32
F32R = mybir.dt.float32r
BF16 = mybir.dt.bfloat16
AF = mybir.ActivationFunctionType
OP = mybir.AluOpType
AX = mybir.AxisListType

# Model dims (hardcoded per the problem spec).
B, L, N, C = 4, 8, 1024, 1024
H, DHEAD = 16, 64
HID = 4 * C
D = 8
EPS = 1e-8

NCORES = 8
T = (B * N) // NCORES          # 512 tokens per core
KC = C // 128                  # 8 feature chunks
TQ = T // 128                  # 4 token tiles
MG = 3                         # matmul m-group size (PSUM banks per group)

DT = BF16                      # matmul compute dtype
DT_NP = ml_dtypes.bfloat16

REPLICA_GROUPS = [[0, 1], [2, 3], [4, 5], [6, 7]]

# oh8 variant j broadcasts den rows (2j, 2j+1) of an [8, T] z tile to the
# (low, high) 64-partition halves: oh8[j][r, p] = (r == 2j + (p >= 64)).
_ONEHOT = np.zeros((8, 4, 128), dtype=np.float32)
for _j in range(4):
    _ONEHOT[2 * _j, _j, 0:64] = 1.0
    _ONEHOT[2 * _j + 1, _j, 64:128] = 1.0
_ONEHOT = np.ascontiguousarray(_ONEHOT.reshape(8, 512))


def _round_f32r(x: np.ndarray) -> np.ndarray:
    """Round fp32 to fp32r (RNE to 11 mantissa bits, low 12 bits zero)."""
    x = np.ascontiguousarray(x, dtype=np.float32)
    u = x.view(np.uint32)
    lo = (u >> 12) & 1
    return ((u + 0x7FF + lo) & 0xFFFFF000).view(np.float32)


def _cast_w(w: np.ndarray) -> np.ndarray:
    if DT is BF16:
        return np.ascontiguousarray(w, dtype=np.float32).astype(DT_NP)
    return _round_f32r(w)


def _slab(dram_ap, k0, kn, n0, nn):
    """DRAM weight slice [k0:k0+kn*128, n0:n0+nn] -> [128, kn, nn] AP."""
    return dram_ap[k0 * 128:(k0 + kn) * 128, n0:n0 + nn].rearrange(
        "(kc p) n -> p kc n", p=128
    )


# ---- host-side weight packing ----
# Weights are re-laid-out on the host into the exact slab stream the device
# DMAs, with each slab [128, kn, nn] stored partition-major-contiguous. This
# turns every weight DMA into 128 long descriptors (one per partition)
# instead of kn*128 short ones, which keeps the SP queue (descriptor issue)
# off the critical path.
G0N = 2 * MG + 2               # bnf1 group-0 width (PSUM banks)


def _mm_plan(mtiles, stat):
    if stat and mtiles == KC:
        # size-1 tail groups: only chunk 7's drain chain is exposed at the
        # LN boundary.
        return [MG, MG, 1, 1]
    plan = [MG] * (mtiles // MG)
    if mtiles % MG:
        plan.append(mtiles % MG)
    return plan


def _mm_requests(kin, mtiles, stat, n_base=0):
    """(k0, kn, n0, nn) slab requests in the device DMA issue order."""
    plan = _mm_plan(mtiles, stat)
    starts = [sum(plan[:i]) for i in range(len(plan))]
    reqs = []
    for g, sz in enumerate(plan):
        nsub = (kin + KC - 1) // KC
        for sb in range(nsub):
            k0, kn = sb * KC, min(KC, kin - sb * KC)
            reqs.append((k0, kn, n_base + starts[g] * 128, sz * 128))
    return reqs


_WREQS = {
    "bn1": [(kc, 1, 0, G0N * 128) for kc in range(KC)]
           + _mm_requests(KC, HID // 128 - G0N, False, n_base=G0N * 128),
    "bn2": _mm_requests(HID // 128, KC, True),
    "qkv": [(k0, kn, base + cg * 512, 512)
            for base in (C, 2 * C) for cg in (0, 1)
            for k0, kn in ((0, 6), (6, 2))]
           + _mm_requests(KC, KC, False),
    "proj": _mm_requests(KC, KC, True),
    "fc1": _mm_requests(KC, HID // 128, False),
    "fc2": _mm_requests(HID // 128, KC, True),
}


def _pack_w(w, reqs):
    """w [K, N] f32 -> 1D slab-stream array (partition-major per slab)."""
    parts = []
    for k0, kn, n0, nn in reqs:
        blk = w[k0 * 128:(k0 + kn) * 128, n0:n0 + nn]
        parts.append(np.ascontiguousarray(
            blk.reshape(kn, 128, nn).transpose(1, 0, 2)).reshape(-1))
    return np.concatenate(parts)


def _wlen(reqs):
    return sum(128 * kn * nn for _, kn, _, nn in reqs)


class _Cursor:
    """Walks a packed 1D weight blob in slab-stream order."""

    def __init__(self, dram_ap):
        self.d = dram_ap
        self.off = 0

    def slab(self, kn, nn):
        sz = 128 * kn * nn
        ap = self.d[self.off:self.off + sz].rearrange(
            "(p k n) -> p k n", p=128, k=kn)
        self.off += sz
        return ap


def build_nc(collectives=True, repeat=1):
    nc = bacc.Bacc("TRN2", target_bir_lowering=False, debug=False)

    en_d = nc.dram_tensor("en", [C, L, T], DT, kind="ExternalInput")
    qkvw_d = nc.dram_tensor("qkvw", [D, _wlen(_WREQS["qkv"])], DT,
                            kind="ExternalInput")
    projw_d = nc.dram_tensor("projw", [D, _wlen(_WREQS["proj"])], DT,
                             kind="ExternalInput")
    fc1w_d = nc.dram_tensor("fc1w", [D, _wlen(_WREQS["fc1"])], DT,
                            kind="ExternalInput")
    fc2w_d = nc.dram_tensor("fc2w", [D, _wlen(_WREQS["fc2"])], DT,
                            kind="ExternalInput")
    bn1w_d = nc.dram_tensor("bn1w", [_wlen(_WREQS["bn1"])], DT,
                            kind="ExternalInput")
    bn2w_d = nc.dram_tensor("bn2w", [_wlen(_WREQS["bn2"])], DT,
                            kind="ExternalInput")
    onehot_d = nc.dram_tensor("onehot", [8, 512], DT, kind="ExternalInput")
    out_d = nc.dram_tensor("out", [4, C, T], F32, kind="ExternalOutput")

    with tile.TileContext(nc) as tc:
        with (
            tc.tile_pool(name="wcg", bufs=4) as wcg,        # weight slabs + en stream
            tc.tile_pool(name="abuf", bufs=1) as abuf,      # block activations
            tc.tile_pool(name="xhp", bufs=2) as xhp,        # normalized inputs
            tc.tile_pool(name="carryp", bufs=2) as carryp,  # residual stream
            tc.tile_pool(name="accp", bufs=1) as accp,      # de accumulators
            tc.tile_pool(name="temps", bufs=4) as temps,    # [128,512] f32 temps
            tc.tile_pool(name="t16", bufs=4) as t16p,       # [128,512] bf16 temps
            tc.tile_pool(name="outs", bufs=1) as outsp,     # en/de map staging
            tc.tile_pool(name="statp", bufs=1) as statp,    # LN stat accums
            tc.tile_pool(name="smalls", bufs=1) as smalls,
            tc.tile_pool(name="consts", bufs=1) as consts,
            tc.tile_pool(name="psA", bufs=2 * MG, space="PSUM") as psA,
            tc.tile_pool(name="psB", bufs=2, space="PSUM") as psB,
            tc.tile_pool(name="dram", bufs=2, space="DRAM") as dramp,
        ):
            # ---- constants ----
            # memset can't write f32r; stage in f32 and tensor_copy (a valid
            # f32r-rounding producer).
            cstf = consts.tile([128, 128], F32, name="cstf")
            nc.vector.memset(cstf, 1.0)
            ones1 = consts.tile([128, 1], F32R, name="ones1")
            nc.vector.tensor_copy(out=ones1, in_=cstf[:, 0:1])
            # 1/sqrt(C)-scaled ones: the sum-stat matmul emits sum/sqrt(C)
            # so s2 = (sum/sqrt(C))^2 = sum^2/C without an extra scale op.
            ones1s = consts.tile([128, 1], F32R, name="ones1s")
            nc.vector.tensor_scalar_mul(ones1s, cstf[:, 0:1], C ** -0.5)
            ones1sb = consts.tile([128, 1], BF16, name="ones1sb")
            nc.vector.tensor_scalar_mul(ones1sb, cstf[:, 0:1], C ** -0.5)
            ones128s = consts.tile([1, 128], F32R, name="ones128s")
            nc.vector.tensor_scalar_mul(ones128s, cstf[0:1, :], C ** -0.5)
            neg1 = consts.tile([1, 1], F32R, name="neg1")
            nc.vector.tensor_scalar_mul(neg1, cstf[0:1, 0:1], -1.0)
            # sqrt(C) row: folds the 1/C variance scale into the rstd
            # broadcast (rpl = sqrt(C)/sqrt(ssq - sum*m + C*eps)).
            rootc = consts.tile([1, 128], F32R, name="rootc")
            nc.vector.tensor_scalar_mul(rootc, cstf[0:1, :], float(C) ** 0.5)
            epsc = consts.tile([1, 1], F32, name="epsc")
            nc.vector.memset(epsc, float(C) * EPS)
            dummy = consts.tile([1, 1], F32, name="dummy")
            oh8 = consts.tile([8, 4, 128], BF16, name="oh8")
            nc.sync.dma_start(out=oh8.rearrange("p j c -> p (j c)"),
                              in_=onehot_d[:, :])

            def one_pass():
                acc0 = accp.tile([128, KC, T], F32, name="acc0")
                acc1 = accp.tile([128, KC, T], F32, name="acc1")
                kvbd = accp.tile([128, KC, 2 * DHEAD], DT, name="kvbd")
                nc.gpsimd.memset(kvbd, 0.0)

                # ---- stage 0: en means, x0 (chunk-streamed so the
                # bottleneck MLP can start on chunk 0 immediately).
                # bnf1's first m-group (6 PSUM banks, per-chunk weight
                # slabs interleaved into the DMA stream) trickles on the PE
                # while the en stream lands; the rest of bnf1 runs dense.
                G0 = 2 * MG + 2
                x0 = xhp.tile([128, KC, T], DT, tag="xh", name="x0")
                hbuf = abuf.tile([128, HID // 128, T], DT, tag="h", name="bn_h")
                g0ps = [psA.tile([128, T], F32, tag="big", name=f"bnf1_g0p{m}")
                        for m in range(2 * MG)]
                g0ps += [psB.tile([128, T], F32, tag="aux", name=f"bnf1_g0q{m}")
                         for m in range(2)]
                bn1_cur = _Cursor(bn1w_d)
                for kc in range(KC):
                    # one 4-layer bf16 slab per half (en is [C, L, T] on
                    # host, so each DMA is 128 contiguous 4KB descriptors);
                    # layer-sum on Pool (half 0, add tree) and DVE (half 1,
                    # X-reduce). The 1/8 layer mean is folded into bn1w
                    # host-side, so x0 is a plain add of the two half-sums.
                    for half, dst in ((0, acc1), (1, acc0)):
                        sl = wcg.tile([128, 4, T], DT, tag="wcg",
                                      name=f"en{kc}{half}")
                        l0 = half * 4
                        # spread DMA issue cost: SP and Act alternate (the
                        # scalar engine is otherwise idle during stage 0)
                        eng = nc.sync if half == 0 else nc.scalar
                        eng.dma_start(
                            out=sl,
                            in_=en_d[kc * 128:(kc + 1) * 128, l0:l0 + 4, :])
                        if half == 0:
                            pa = temps.tile([128, T], F32, tag="tmp",
                                            name=f"en{kc}{half}a")
                            nc.gpsimd.tensor_tensor(pa, sl[:, 0, :],
                                                    sl[:, 1, :], op=OP.add)
                            pb = temps.tile([128, T], F32, tag="tmp",
                                            name=f"en{kc}{half}b")
                            nc.gpsimd.tensor_tensor(pb, sl[:, 2, :],
                                                    sl[:, 3, :], op=OP.add)
                            nc.gpsimd.tensor_tensor(dst[:, kc, :], pa, pb,
                                                    op=OP.add)
                        else:
                            nc.vector.tensor_reduce(
                                out=dst[:, kc, :],
                                in_=sl.rearrange("p l t -> p t l"),
                                axis=AX.X, op=OP.add)
                    nc.vector.tensor_tensor(x0[:, kc, :], acc0[:, kc, :],
                                            acc1[:, kc, :], op=OP.add)
                    wt = wcg.tile([128, 1, G0 * 128], DT, tag="wcg",
                                  name=f"bnf1_g0w{kc}")
                    nc.scalar.dma_start(out=wt, in_=bn1_cur.slab(1, G0 * 128))
                    for m in range(G0):
                        nc.tensor.matmul(
                            g0ps[m], wt[:, 0, m * 128:(m + 1) * 128],
                            x0[:, kc, :],
                            start=(kc == 0), stop=(kc == KC - 1))

                # ---- helpers ----
                def mm_layer(cur, rhs_tile, kin, mtiles, out_cb, name,
                             stat_cb=None, tail=False):
                    """out[m] = sum_k W[k, m].T @ rhs[k]; m-grouped, K-accum.

                    cur: _Cursor over the layer's packed slab stream;
                    rhs_tile [128, kin, T]; out_cb(mi, psum_tile) consumes
                    each finished [128, T] output. stat_cb(mi) may emit PE
                    work that reads out_cb's result; it is lagged one group
                    so the PE never waits on the drain. `tail` must match
                    the host-side _WREQS plan for this layer type (it can't
                    follow stat_cb: block 7's fc2 has no stats but shares
                    the packed fc2 layout).
                    """
                    plan = _mm_plan(mtiles, tail)
                    starts = [sum(plan[:i]) for i in range(len(plan))]
                    prev_stat = []
                    for g in range(len(plan)):
                        ms = [starts[g] + i for i in range(plan[g])]
                        nsub = (kin + KC - 1) // KC
                        pss = {}
                        for m in ms:
                            pss[m] = psA.tile([128, T], F32, tag="big",
                                              name=f"{name}_ps{m}")
                        for sb in range(nsub):
                            k0, kn = sb * KC, min(KC, kin - sb * KC)
                            wt = wcg.tile([128, kn, len(ms) * 128], DT,
                                          tag="wcg", name=f"{name}_w{g}_{sb}")
                            nc.sync.dma_start(
                                out=wt, in_=cur.slab(kn, len(ms) * 128))
                            for kc in range(kn):
                                for j, m in enumerate(ms):
                                    nc.tensor.matmul(
                                        pss[m],
                                        wt[:, kc, j * 128:(j + 1) * 128],
                                        rhs_tile[:, k0 + kc, :],
                                        start=(sb == 0 and kc == 0),
                                        stop=(sb == nsub - 1 and kc == kn - 1),
                                    )
                        for m in prev_stat:
                            stat_cb(m)
                        for m in ms:
                            out_cb(m, pss[m])
                        if stat_cb is not None:
                            prev_stat = ms
                    for m in prev_stat:
                        stat_cb(m)

                # LayerNorm: per-chunk Squares run on Act as the producer
                # emits each carry chunk; chunks 0-5 pre-accumulate on DVE
                # (sums) and Pool (squares) so only 3 sum + 3 ssq matmuls
                # hit the PE per LN. The sum matmul is 1/sqrt(C)-scaled so
                # sum^2/C needs no extra scale op; the variance is closed
                # by a K=1 matmul accumulating -s2 into the ssq PSUM group.
                def ln_stats(name):
                    return {
                        "sum": psB.tile([1, T], F32, tag="aux", name=f"{name}_sum"),
                        "ssq": psB.tile([1, T], F32, tag="aux", name=f"{name}_ssq"),
                        "sacc": statp.tile([128, T], BF16, tag="sacc",
                                           name=f"{name}_sacc"),
                        "sqacc": statp.tile([128, T], F32R, tag="sqacc",
                                            name=f"{name}_sqacc"),
                        "c0": None, "sq0": None,
                    }

                def ln_chunk(st, src, kc, name):
                    sq = temps.tile([128, T], F32R, tag="tmp",
                                    name=f"{name}_sq{kc}")
                    nc.scalar.activation(out=sq, in_=src, func=AF.Square)
                    if kc == 0:
                        st["c0"], st["sq0"] = src, sq
                    elif kc == 1:
                        nc.vector.tensor_tensor(st["sacc"], st["c0"], src,
                                                op=OP.add)
                        nc.gpsimd.tensor_tensor(st["sqacc"], st["sq0"], sq,
                                                op=OP.add)
                    elif kc <= 5:
                        nc.vector.tensor_tensor(st["sacc"], st["sacc"], src,
                                                op=OP.add)
                        nc.gpsimd.tensor_tensor(st["sqacc"], st["sqacc"], sq,
                                                op=OP.add)
                    elif kc == 6:
                        nc.tensor.matmul(st["sum"], ones1sb, st["sacc"],
                                         start=True, stop=False)
                        nc.tensor.matmul(st["sum"], ones1s, src,
                                         start=False, stop=False)
                        nc.tensor.matmul(st["ssq"], ones1, st["sqacc"],
                                         start=True, stop=False)
                        nc.tensor.matmul(st["ssq"], ones1, sq,
                                         start=False, stop=False)
                    else:  # kc == 7; ssq group is closed by ln_finish's -s2 MM
                        nc.tensor.matmul(st["sum"], ones1s, src,
                                         start=False, stop=True)
                        nc.tensor.matmul(st["ssq"], ones1, sq,
                                         start=False, stop=False)

                def ln_finish(st, carry, xh, name, next_func=None):
                    # rstd' = 1/sqrt(ssq - sum^2/C + C*eps); the missing
                    # sqrt(C) rides on the rpl broadcast (rootc).
                    m_sb = smalls.tile([1, T], F32R, tag="m_sb", name=f"{name}_m")
                    nc.vector.tensor_copy(out=m_sb, in_=st["sum"])
                    s2 = temps.tile([1, T], F32R, tag="tmp", name=f"{name}_s2")
                    nc.scalar.activation(out=s2, in_=st["sum"], func=AF.Square)
                    nc.tensor.matmul(st["ssq"], neg1, s2, start=False, stop=True)
                    sd = temps.tile([1, T], F32, tag="tmp", name=f"{name}_sd")
                    nc.scalar.activation(out=sd, in_=st["ssq"], func=AF.Sqrt,
                                         bias=epsc)
                    rstd = smalls.tile([1, T], F32R, tag="rstd",
                                       name=f"{name}_rstd")
                    with nc.allow_low_precision(reason="f32r feeds PE bcast"):
                        nc.vector.reciprocal(out=rstd, in_=sd)
                    if next_func is not None:
                        # prefetch the next act-table set (exp/gelu) right
                        # after the Rsqrt so the load overlaps the GEMM
                        nc.scalar.activation(out=dummy, in_=rstd[0:1, 0:1],
                                             func=next_func)
                    ps_mpl = psB.tile([128, T], F32, tag="aux", name=f"{name}_mpl")
                    nc.tensor.matmul(ps_mpl, ones128s, m_sb,
                                     start=True, stop=True)
                    ps_rpl = psB.tile([128, T], F32, tag="aux", name=f"{name}_rpl")
                    nc.tensor.matmul(ps_rpl, rootc, rstd, start=True, stop=True)
                    for kc in range(KC):
                        t1 = temps.tile([128, T], F32, tag="tmp",
                                        name=f"{name}_c{kc}")
                        nc.vector.tensor_tensor(t1, carry[:, kc, :], ps_mpl,
                                                op=OP.subtract)
                        nc.vector.tensor_tensor(xh[:, kc, :], t1, ps_rpl,
                                                op=OP.mult)

                def elu1(ps_in, out_ap, name, prefetch=None):
                    """out = elu(x)+1 = exp(min(x,0)) + relu(x), from PSUM.

                    relu(-x) on Act (scale=-1) feeds exp(-.) so the DVE only
                    pays one fused (max,add) scalar_tensor_tensor."""
                    rn = t16p.tile([128, T], BF16, tag="t16", name=f"{name}_rn")
                    nc.scalar.activation(out=rn, in_=ps_in, func=AF.Relu,
                                         scale=-1.0)
                    e = t16p.tile([128, T], BF16, tag="t16", name=f"{name}_e")
                    nc.scalar.activation(out=e, in_=rn, func=AF.Exp, scale=-1.0)
                    if prefetch is not None:
                        nc.scalar.activation(out=dummy, in_=e[0:1, 0:1],
                                             func=prefetch, scale=0.0,
                                             bias=epsc)
                    nc.vector.scalar_tensor_tensor(out_ap, ps_in, 0.0, e,
                                                   op0=OP.max, op1=OP.add)

                # ---- stage 1: bottleneck MLP ----
                def bn_gelu(m, ps):
                    nc.scalar.activation(out=hbuf[:, m, :], in_=ps, func=AF.Gelu)
                    if m == HID // 128 - 1:
                        nc.scalar.activation(out=dummy, in_=hbuf[0:1, m, 0:1],
                                             func=AF.Sqrt, scale=0.0, bias=epsc)

                for m in range(G0):
                    bn_gelu(m, g0ps[m])
                mm_layer(bn1_cur, x0, KC, HID // 128 - G0,
                         lambda m, ps: bn_gelu(m + G0, ps), "bnf1")

                # en map staging late: keeps the out DMAs off the stage-0
                # DMA stream (acc0/acc1 stay valid until blocks 0/4).
                for kc in range(KC):
                    for i, acc in ((0, acc1), (1, acc0)):
                        st = outsp.tile([128, T], F32, tag="outst",
                                        name=f"en{i}st{kc}")
                        nc.gpsimd.tensor_scalar_mul(st, acc[:, kc, :], 0.25)
                        nc.gpsimd.dma_start(
                            out=_slab(out_d[i], kc, 1, 0, T), in_=st)

                carry = carryp.tile([128, KC, T], F32R, tag="carry",
                                    name="carry_bn")
                stats = ln_stats("b0ln1")

                def bn_out(m, ps, _c=carry):
                    nc.vector.tensor_copy(out=_c[:, m, :], in_=ps)

                def bn_stat(m, _st=stats, _c=carry):
                    ln_chunk(_st, _c[:, m, :], m, "b0ln1")

                mm_layer(_Cursor(bn2w_d), hbuf, HID // 128, KC, bn_out,
                         "bnf2", stat_cb=bn_stat, tail=True)

                # ---- stage 2: decoder blocks ----
                for d in range(D):
                    xh = xhp.tile([128, KC, T], DT, tag="xh", name=f"b{d}_xh")
                    ln_finish(stats, carry, xh, f"b{d}ln1", next_func=AF.Exp)

                    # k, v token-major: out[t, feat] tiles [128, 512]
                    kT = abuf.tile([128, TQ, C], DT, tag="kT", name=f"b{d}_kT")
                    vA = abuf.tile([128, TQ, H, DHEAD + 1], DT, tag="vA",
                                   name=f"b{d}_vA")
                    nc.vector.memset(vA[:, :, :, DHEAD:DHEAD + 1], 1.0)
                    qkv_cur = _Cursor(qkvw_d[d])

                    def kv_slabs(nm, _cur=qkv_cur):
                        # split 8-chunk slab 6+2 so the wcg slot stays 6KB
                        wta = wcg.tile([128, 6, 512], DT, tag="wcg",
                                       name=f"{nm}a")
                        nc.sync.dma_start(out=wta, in_=_cur.slab(6, 512))
                        wtb = wcg.tile([128, 2, 512], DT, tag="wcg",
                                       name=f"{nm}b")
                        nc.sync.dma_start(out=wtb, in_=_cur.slab(2, 512))
                        return wta, wtb

                    def kv_mm(ps, wta, wtb, tt):
                        for kc in range(KC):
                            wt_ap = (wta[:, kc, :] if kc < 6
                                     else wtb[:, kc - 6, :])
                            nc.tensor.matmul(
                                ps, xh[:, kc, tt * 128:(tt + 1) * 128], wt_ap,
                                start=(kc == 0), stop=(kc == KC - 1))

                    for cg in range(2):       # two 512-col groups of k feats
                        wta, wtb = kv_slabs(f"b{d}_wk{cg}")
                        for tt in range(TQ):
                            ps = psA.tile([128, 512], F32, tag="big",
                                          name=f"b{d}_psk{cg}{tt}")
                            kv_mm(ps, wta, wtb, tt)
                            elu1(ps, kT[:, tt, cg * 512:(cg + 1) * 512],
                                 f"b{d}ek{cg}{tt}")
                    for cg in range(2):       # v
                        wta, wtb = kv_slabs(f"b{d}_wv{cg}")
                        for tt in range(TQ):
                            ps = psA.tile([128, 512], F32, tag="big",
                                          name=f"b{d}_psv{cg}{tt}")
                            kv_mm(ps, wta, wtb, tt)
                            nc.scalar.copy(
                                out=vA[:, tt, cg * 8:(cg + 1) * 8, 0:DHEAD],
                                in_=ps.rearrange("p (h e) -> p h e", h=8))

                    # kv partial: per head [64, 65], packed in pairs on
                    # partitions
                    kvps = [psB.tile([128, 4, 128], F32, tag="aux",
                                     name=f"b{d}_kv{i}") for i in range(2)]
                    for h in range(H):
                        j, p = h // 2, 64 * (h % 2)
                        ps = kvps[j // 4]
                        for tt in range(TQ):
                            nc.tensor.matmul(
                                ps[p:p + 64, j % 4, 0:DHEAD + 1],
                                kT[:, tt, h * 64:h * 64 + 64],
                                vA[:, tt, h, :],
                                start=(tt == 0), stop=(tt == TQ - 1))
                    kvp = smalls.tile([128, 8, DHEAD + 1], F32, tag="kvp",
                                      name=f"b{d}_kvp")
                    nc.scalar.copy(out=kvp[:, 0:4, :],
                                   in_=kvps[0][:, :, 0:DHEAD + 1])
                    nc.scalar.copy(out=kvp[:, 4:8, :],
                                   in_=kvps[1][:, :, 0:DHEAD + 1])

                    ar_in = dramp.tile([128, 8 * (DHEAD + 1)], F32, tag="arin",
                                       name=f"b{d}_arin")
                    ar_out = dramp.tile([128, 8 * (DHEAD + 1)], F32,
                                        tag="arout", name=f"b{d}_arout")
                    nc.sync.dma_start(out=ar_in,
                                      in_=kvp.rearrange("p a b -> p (a b)"))
                    if collectives:
                        nc.gpsimd.collective_compute(
                            "AllReduce", OP.add,
                            ins=[ar_in.opt()], outs=[ar_out.opt()],
                            replica_groups=REPLICA_GROUPS)
                    else:
                        nc.sync.dma_start(out=ar_out, in_=ar_in)
                    kvf = smalls.tile([128, 8, DHEAD + 1], F32, tag="kvf",
                                      name=f"b{d}_kvf")
                    nc.sync.dma_start(out=kvf.rearrange("p a b -> p (a b)"),
                                      in_=ar_out)
                    # block-diag kv per chunk: one K=128 attn MM per
                    # chunk instead of two 64-partition halves. The zero
                    # off-diagonal blocks are memset once per pass.
                    nc.gpsimd.tensor_copy(out=kvbd[0:64, :, 0:DHEAD],
                                          in_=kvf[0:64, :, 0:DHEAD])
                    nc.gpsimd.tensor_copy(out=kvbd[64:128, :, DHEAD:2 * DHEAD],
                                          in_=kvf[64:128, :, 0:DHEAD])

                    # q (feature-major) while the AllReduce is in flight
                    qe = abuf.tile([128, KC, T], DT, tag="qe", name=f"b{d}_qe")

                    def q_elu(m, ps, _d=d):
                        elu1(ps, qe[:, m, :], f"b{_d}eq{m}",
                             prefetch=AF.Sqrt if m == KC - 1 else None)

                    mm_layer(qkv_cur, xh, KC, KC, q_elu, f"b{d}q")

                    # block-diag ksum for den: 4 chunks accumulate into one
                    # [8, T] den tile (zero-padded bd8 columns keep the out
                    # base at 0), so z needs 2 reciprocals, not 8. zb then
                    # broadcasts rows (2j, 2j+1) via the oh8 constants.
                    bd8 = smalls.tile([128, KC, 8], DT, tag="bd", name=f"b{d}_bd")
                    nc.gpsimd.memset(bd8, 0.0)
                    for c in range(KC):
                        for e in range(2):
                            p = 64 * e
                            nc.gpsimd.tensor_copy(
                                out=bd8[p:p + 64, c, 2 * (c % 4) + e:
                                        2 * (c % 4) + e + 1],
                                in_=kvf[p:p + 64, c, DHEAD:DHEAD + 1])
                    z8s = []
                    for h in range(2):
                        psden = psB.tile([8, T], F32, tag="aux",
                                         name=f"b{d}_den{h}")
                        for i in range(4):
                            c = 4 * h + i
                            nc.tensor.matmul(psden, bd8[:, c, :], qe[:, c, :],
                                             start=(i == 0), stop=(i == 3))
                        z8 = t16p.tile([8, T], BF16, tag="t16",
                                       name=f"b{d}_z{h}")
                        with nc.allow_low_precision(reason="f32r PE bcast"):
                            nc.vector.reciprocal(out=z8, in_=psden)
                        z8s.append(z8)
                    # attention out per head -> attn_sb (feature-major);
                    # qz = qe * zb (one PSUM operand - walrus rejects dual-
                    # PSUM TensorTensor), then one block-diag K=128 attn MM
                    # per chunk.
                    qz = abuf.tile([128, KC, T], DT, tag="qz", name=f"b{d}_qz")
                    attn_sb = abuf.tile([128, KC, T], DT, tag="attn",
                                        name=f"b{d}_attn")
                    for j in range(KC):
                        pszb = psB.tile([128, T], F32, tag="aux",
                                        name=f"b{d}_zb{j}")
                        nc.tensor.matmul(pszb, oh8[:, j % 4, :], z8s[j // 4],
                                         start=True, stop=True)
                        nc.vector.tensor_tensor(qz[:, j, :], qe[:, j, :], pszb,
                                                op=OP.mult)
                        psat = psA.tile([128, T], F32, tag="big",
                                        name=f"b{d}_at{j}")
                        nc.tensor.matmul(psat, kvbd[:, j, :], qz[:, j, :],
                                         start=True, stop=True)
                        nc.scalar.copy(out=attn_sb[:, j, :], in_=psat)

                    # proj + residual; ln2 stats accumulate as chunks land
                    carry2 = carryp.tile([128, KC, T], F32R, tag="carry",
                                         name=f"b{d}_carry2")
                    stats2 = ln_stats(f"b{d}ln2")

                    def proj_out(m, ps, _c=carry, _c2=carry2):
                        nc.vector.tensor_tensor(_c2[:, m, :], _c[:, m, :], ps,
                                                op=OP.add)

                    def proj_stat(m, _c2=carry2, _st=stats2, _d=d):
                        ln_chunk(_st, _c2[:, m, :], m, f"b{_d}ln2")

                    mm_layer(_Cursor(projw_d[d]), attn_sb, KC, KC,
                             proj_out, f"b{d}pr", stat_cb=proj_stat, tail=True)

                    # mlp
                    xh2 = xhp.tile([128, KC, T], DT, tag="xh", name=f"b{d}_xh2")
                    ln_finish(stats2, carry2, xh2, f"b{d}ln2", next_func=AF.Gelu)
                    hb = abuf.tile([128, HID // 128, T], DT, tag="h",
                                   name=f"b{d}_h")

                    def mlp_gelu(m, ps, _h=hb, _last=(d < D - 1)):
                        nc.scalar.activation(out=_h[:, m, :], in_=ps, func=AF.Gelu)
                        if _last and m == HID // 128 - 1:
                            nc.scalar.activation(out=dummy, in_=_h[0:1, m, 0:1],
                                                 func=AF.Sqrt, scale=0.0,
                                                 bias=epsc)

                    mm_layer(_Cursor(fc1w_d[d]), xh2, KC, HID // 128,
                             mlp_gelu, f"b{d}f1")

                    carry3 = carryp.tile([128, KC, T], F32R, tag="carry",
                                         name=f"b{d}_carry3")
                    accd = acc1 if d < 4 else acc0
                    stats = ln_stats(f"b{d + 1}ln1") if d < D - 1 else None
                    # de1 = acc1/4 is final after block 3; de0 = acc0/4 after
                    # block 7 -> stream the map out as its chunks finalize.
                    emit = {3: (3, acc1), 7: (2, acc0)}.get(d)

                    def mlp_out(m, ps, _c2=carry2, _c3=carry3, _a=accd,
                                _em=emit, _first=(d == 0 or d == 4)):
                        nc.vector.tensor_tensor(_c3[:, m, :], _c2[:, m, :], ps,
                                                op=OP.add)
                        if _first:
                            nc.gpsimd.tensor_copy(out=_a[:, m, :],
                                                  in_=_c3[:, m, :])
                        else:
                            nc.gpsimd.tensor_tensor(_a[:, m, :], _a[:, m, :],
                                                    _c3[:, m, :], op=OP.add)
                        if _em is not None:
                            oi, acc = _em
                            st_t = outsp.tile([128, T], F32, tag="outst",
                                              name=f"de{oi}st{m}")
                            nc.gpsimd.tensor_scalar_mul(st_t, acc[:, m, :], 0.25)
                            nc.gpsimd.dma_start(
                                out=_slab(out_d[oi], m, 1, 0, T), in_=st_t)

                    def mlp_stat(m, _c3=carry3, _st=stats, _d=d):
                        if _st is not None:
                            ln_chunk(_st, _c3[:, m, :], m, f"b{_d + 1}ln1")

                    mm_layer(_Cursor(fc2w_d[d]), hb, HID // 128, KC,
                             mlp_out, f"b{d}f2",
                             stat_cb=mlp_stat if d < D - 1 else None,
                             tail=True)
                    carry = carry3

            for _rep in range(repeat):
                one_pass()

    nc.compile()
    return nc


_NC_CACHE = None


def kernel(**inputs) -> np.ndarray:
    global _NC_CACHE
    en_feats = np.asarray(inputs["en_feats"], dtype=np.float32)

    # Fold LayerNorm affine params into the following matmul weights (host-side
    # preprocessing of replicated params). Biases in this module are all zero;
    # verify and skip them on device.
    for bname in ("bn_fc1_b", "bn_fc2_b", "qkv_b", "proj_b", "mlp_fc1_b",
                  "mlp_fc2_b", "ln1_b", "ln2_b"):
        assert np.abs(np.asarray(inputs[bname])).max() == 0.0, bname
    ln1_w = np.asarray(inputs["ln1_w"], dtype=np.float32)
    ln2_w = np.asarray(inputs["ln2_w"], dtype=np.float32)
    qkvw = np.asarray(inputs["qkv_w"], dtype=np.float32) * ln1_w[:, :, None]
    fc1w = np.asarray(inputs["mlp_fc1_w"], dtype=np.float32) * ln2_w[:, :, None]

    projw = np.asarray(inputs["proj_w"], dtype=np.float32)
    fc2w = np.asarray(inputs["mlp_fc2_w"], dtype=np.float32)
    wmap = {
        "qkvw": np.stack([_pack_w(_cast_w(qkvw[d]), _WREQS["qkv"])
                          for d in range(D)]),
        "projw": np.stack([_pack_w(_cast_w(projw[d]), _WREQS["proj"])
                           for d in range(D)]),
        "fc1w": np.stack([_pack_w(_cast_w(fc1w[d]), _WREQS["fc1"])
                          for d in range(D)]),
        "fc2w": np.stack([_pack_w(_cast_w(fc2w[d]), _WREQS["fc2"])
                          for d in range(D)]),
        # 1/L layer-mean folded into bn1w (device x0 is the plain layer sum)
        "bn1w": _pack_w(_cast_w(np.asarray(inputs["bn_fc1_w"],
                                           dtype=np.float32) / L),
                        _WREQS["bn1"]),
        "bn2w": _pack_w(_cast_w(np.asarray(inputs["bn_fc2_w"],
                                           dtype=np.float32)),
                        _WREQS["bn2"]),
    }

    in_maps = []
    for c in range(NCORES):
        b, hf = c // 2, c % 2
        sl = en_feats[b, :, hf * T:(hf + 1) * T, :]          # [L, T, C]
        en_c = np.ascontiguousarray(sl.transpose(2, 0, 1))   # [C, L, T]
        in_maps.append({"en": en_c.astype(DT_NP), **wmap,
                        "onehot": _ONEHOT.astype(DT_NP)})

    if _NC_CACHE is None:
        _NC_CACHE = build_nc()
    nc = _NC_CACHE

    trace = os.environ.get("BASS_KERNEL_TRACE", "0") == "1"
    res = run_bass_kernel_spmd(nc, in_maps, core_ids=list(range(NCORES)),
                               trace=trace)
    if trace and res.exec_time_ns is not None:
        print(f"HW exec time: {res.exec_time_ns} ns")
        if res.instructions_and_trace is not None:
            print(f"trace: {res.instructions_and_trace[1]}")

    out = np.empty((4, B, C, N), dtype=np.float32)
    for c in range(NCORES):
        b, hf = c // 2, c % 2
        out[:, b, :, hf * T:(hf + 1) * T] = res.results[c]["out"]
    return out.reshape(4, B, C, 32, 32)

